# revision 1
# baseline (speedup 1.0000x reference)
"""Position Attention Module (DANet) on 8 Trainium2 NeuronCores.

Reference computation (per batch b of 4):
  xf = x[b] : [C=512, N=4096]
  q = Wq@xf + bq : [64, N];  k = Wk@xf + bk : [64, N];  v = Wv@xf + bv : [512, N]
  scores[i,j] = q[:,i].k[:,j];  attn = softmax_j(scores)
  out[c,i] = alpha * sum_j v[c,j] attn[i,j]

Sharding: 2 cores per batch, each core owns half the query rows (i), full k/v.
Per-core x is pre-rolled on host so the owned i-half is always columns 0:2048.

Device design:
  - alpha folded into Wv/bv on the host.
  - q/k projections in plain fp32 (precision feeds exp; f32r rounding of q/k
    would inject ~5e-3 absolute score error -> ~1e-3 output error).
  - scoresT [j, i] via K-stacked f32r hi/lo decomposition: KHL = [k_hi; k_lo]
    (128 contraction rows), scores = KHL.[q_hi;q_hi] + KHL.[q_lo;q_lo]
    = k.q_hi + k.q_lo = k.q -- two full-K=128 f32r matmuls, ~fp32 precision.
    (tile_position packing with PSUM accumulation crashes the exec unit.)
  - exp without max-subtraction: scores ~ N(0,64); |s|max ~ 56 << 88.
  - vT produced transposed by the projection (host-transposed Wv), f32r.
  - out [c, i] = vT.T @ expT accumulated over j in PSUM (f32r, fp32 accum).
  - softmax denominator: DVE-accumulated exp sums; ones[128,128] matmul
    broadcasts the partition-sum to all lanes; DVE reciprocal; fused scaling.
  Expected output error ~1.1e-4 (floor set by f32r rounding of v).
"""
import numpy as np


def _round_f32r(a):
    """Round fp32 mantissa to 11 bits (f32r / E8M11), round-half-even."""
    m, e = np.frexp(np.asarray(a, np.float32).astype(np.float64))
    return np.ldexp(np.round(np.ldexp(m, 12)), -12).astype(np.float32) * (2.0 ** e).astype(np.float32)


B, C, HW = 4, 512, 4096
CQ = 64
NCORES = 8
IH = HW // 2          # 2048 query rows per core
ITILE = 512           # i-tile (psum free dim)
NITILES = IH // ITILE # 4
JT = 128              # j-tile (contraction chunk for AV / scores lhsT cols)
NJT = HW // JT        # 32
JB = 512              # j-block for projections
NJB = HW // JB        # 8
NCC = C // 128        # 4 contraction chunks of 128 over C

_cache = {}


def _build():
    import concourse.bacc as bacc
    import concourse.tile as tile
    import concourse.mybir as mybir
    from concourse.bass_utils import run_bass_kernel_spmd

    f32 = mybir.dt.float32
    f32r = mybir.dt.float32r
    bf16 = mybir.dt.bfloat16
    AF = mybir.ActivationFunctionType

    nc = bacc.Bacc("TRN2", target_bir_lowering=False, debug=False)

    x_d = nc.dram_tensor("x", [C, HW], f32, kind="ExternalInput")
    xr_d = nc.dram_tensor("xr", [C, HW], f32, kind="ExternalInput")
    wqt_d = nc.dram_tensor("wqt", [C, CQ], f32, kind="ExternalInput")
    wkt_d = nc.dram_tensor("wkt", [C, CQ], f32, kind="ExternalInput")
    wvt_d = nc.dram_tensor("wvt", [C, C], f32, kind="ExternalInput")
    bq_d = nc.dram_tensor("bq", [CQ, 1], f32, kind="ExternalInput")
    bk_d = nc.dram_tensor("bk", [CQ, 1], f32, kind="ExternalInput")
    bv_d = nc.dram_tensor("bv", [1, C], f32, kind="ExternalInput")
    out_d = nc.dram_tensor("out", [C, IH], f32, kind="ExternalOutput")

    with tile.TileContext(nc) as tc:
        with (
            tc.tile_pool(name="const", bufs=1) as cpool,
            tc.tile_pool(name="kq", bufs=1) as kqpool,
            tc.tile_pool(name="vt", bufs=1) as vtpool,
        ):
            # --- constants / weights ---
            wqt = [cpool.tile([128, CQ], f32, tag=f"wqt{i}", name=f"wqt{i}") for i in range(NCC)]
            wkt = [cpool.tile([128, CQ], f32, tag=f"wkt{i}", name=f"wkt{i}") for i in range(NCC)]
            wvt = [cpool.tile([128, C], f32r, tag=f"wvt{i}", name=f"wvt{i}") for i in range(NCC)]
            for cc in range(NCC):
                sl = slice(cc * 128, (cc + 1) * 128)
                nc.sync.dma_start(wqt[cc][:], wqt_d[sl, :])
                nc.sync.dma_start(wkt[cc][:], wkt_d[sl, :])
                nc.sync.dma_start(wvt[cc][:], wvt_d[sl, :].bitcast(f32r))
            bq_c = cpool.tile([CQ, 1], f32, tag="bqc")
            bk_c = cpool.tile([CQ, 1], f32, tag="bkc")
            nc.sync.dma_start(bq_c[:], bq_d[:])
            nc.sync.dma_start(bk_c[:], bk_d[:])
            bv_row = cpool.tile([1, C], f32, tag="bvrow")
            nc.sync.dma_start(bv_row[:], bv_d[:])
            ones_r = cpool.tile([1, 128], f32, tag="onesr")    # K=1 bcast lhsT
            nc.vector.memset(ones_r[:], 1.0)
            ones_sq = cpool.tile([128, 128], f32, tag="onessq")  # sum+bcast lhsT
            nc.vector.memset(ones_sq[:], 1.0)

            # f32r hi/lo split activations for scores (K-stacked):
            #  KHL [128, HW]: rows 0-63 = k_hi, rows 64-127 = k_lo
            #  QHH [128, IH]: q_hi duplicated on both halves; QLL: q_lo dup
            KHL = kqpool.tile([128, HW], f32r, tag="khl")
            QHH = kqpool.tile([128, IH], f32r, tag="qhh")
            QLL = kqpool.tile([128, IH], f32r, tag="qll")
            vts = [vtpool.tile([JT, C], f32r, tag=f"vt{j}", name=f"vt{j}") for j in range(NJT)]

            # bvB: (alpha*bv) broadcast to 128 partitions (for vT psum eviction)
            with tc.tile_pool(name="ppre", bufs=1, space="PSUM") as ppre:
                bvB = cpool.tile([128, C], f32, tag="bvB")
                ps = ppre.tile([128, C], f32, tag="bvps")
                nc.tensor.matmul(ps[:], ones_r[:], bv_row[:], start=True, stop=True)
                nc.vector.tensor_copy(bvB[:], ps[:])

            # ---------------- projections ----------------
            with (
                tc.tile_pool(name="xin", bufs=8) as xpool,
                tc.tile_pool(name="evt", bufs=3) as evpool,
                tc.tile_pool(name="pkq", bufs=2, space="PSUM") as pkq,
                tc.tile_pool(name="pvt", bufs=3, space="PSUM") as pvt,
            ):
                for jb in range(NJB):
                    jsl = slice(jb * JB, (jb + 1) * JB)
                    xt, xrt = [], []
                    for cc in range(NCC):
                        csl = slice(cc * 128, (cc + 1) * 128)
                        t = xpool.tile([128, JB], f32, tag="x", name=f"x{jb}_{cc}")
                        nc.sync.dma_start(t[:], x_d[csl, jsl])
                        xt.append(t)
                        tr = xpool.tile([128, JB], f32r, tag="xr", name=f"xr{jb}_{cc}")
                        nc.sync.dma_start(tr[:], xr_d[csl, jsl].bitcast(f32r))
                        xrt.append(tr)
                    # k (and q for the owned half) : fp32 matmuls [64, JB]
                    kp = pkq.tile([CQ, JB], f32, tag="kqp")
                    for cc in range(NCC):
                        nc.tensor.matmul(kp[:], wkt[cc][:], xt[cc][:],
                                         start=(cc == 0), stop=(cc == NCC - 1))
                    ktmp = evpool.tile([CQ, JB], f32, tag="ev")
                    nc.scalar.activation(ktmp[:], kp[:], AF.Identity, bias=bk_c[:])
                    nc.vector.tensor_copy(KHL[0:CQ, jsl], ktmp[:])
                    klo = evpool.tile([CQ, JB], f32r, tag="evlo")
                    nc.vector.tensor_sub(klo[:], ktmp[:], KHL[0:CQ, jsl])
                    nc.sync.dma_start(KHL[CQ:128, jsl], klo[:])
                    if jb < NJB // 2:
                        qp = pkq.tile([CQ, JB], f32, tag="kqp")
                        for cc in range(NCC):
                            nc.tensor.matmul(qp[:], wqt[cc][:], xt[cc][:],
                                             start=(cc == 0), stop=(cc == NCC - 1))
                        qtmp = evpool.tile([CQ, JB], f32, tag="ev")
                        nc.scalar.activation(qtmp[:], qp[:], AF.Identity, bias=bq_c[:])
                        nc.vector.tensor_copy(QHH[0:CQ, jsl], qtmp[:])
                        nc.sync.dma_start(QHH[CQ:128, jsl], QHH[0:CQ, jsl])
                        nc.vector.tensor_sub(QLL[0:CQ, jsl], qtmp[:], QHH[0:CQ, jsl])
                        nc.sync.dma_start(QLL[CQ:128, jsl], QLL[0:CQ, jsl])
                    # vT tiles [128 j, C] in f32r
                    for js in range(JB // JT):
                        vp = pvt.tile([JT, C], f32, tag="vtp")
                        for cc in range(NCC):
                            nc.tensor.matmul(
                                vp[:], xrt[cc][:, js * JT:(js + 1) * JT], wvt[cc][:],
                                start=(cc == 0), stop=(cc == NCC - 1))
                        nc.vector.tensor_add(vts[jb * 4 + js][:], vp[:], bvB[:])

            # ---------------- attention ----------------
            with (
                tc.tile_pool(name="expp", bufs=3) as epool,
                tc.tile_pool(name="dnm", bufs=2) as dpool,
                tc.tile_pool(name="ost", bufs=8) as opool,
                tc.tile_pool(name="rows", bufs=2) as rpool,
                tc.tile_pool(name="pso", bufs=2, space="PSUM") as pso,
                tc.tile_pool(name="pout", bufs=5, space="PSUM") as pout,
                tc.tile_pool(name="paux", bufs=1, space="PSUM") as paux,
            ):
                for it in range(NITILES):
                    isl = slice(it * ITILE, (it + 1) * ITILE)
                    ops = [pout.tile([128, ITILE], f32, tag="op", name=f"op{it}_{i}") for i in range(NCC)]
                    dnm = dpool.tile([128, ITILE], f32, tag="dn")
                    for j in range(NJT):
                        jsl = slice(j * JT, (j + 1) * JT)
                        sp = pso.tile([JT, ITILE], f32, tag="sc")
                        nc.tensor.matmul(sp[:], KHL[:, jsl], QHH[:, isl],
                                         start=True, stop=False)
                        nc.tensor.matmul(sp[:], KHL[:, jsl], QLL[:, isl],
                                         start=False, stop=True)
                        et = epool.tile([JT, ITILE], f32r, tag="exp")
                        nc.scalar.activation(et[:], sp[:], AF.Exp)
                        if j == 0:
                            nc.vector.tensor_copy(dnm[:], et[:])
                        else:
                            nc.vector.tensor_add(dnm[:], dnm[:], et[:])
                        for cc in range(NCC):
                            nc.tensor.matmul(
                                ops[cc][:], vts[j][:, cc * 128:(cc + 1) * 128], et[:],
                                start=(j == 0), stop=(j == NJT - 1))
                    # denomB = column-sums of dnm broadcast to all 128 partitions
                    dB = paux.tile([128, ITILE], f32, tag="aux")
                    nc.tensor.matmul(dB[:], ones_sq[:], dnm[:], start=True, stop=True)
                    recipB = rpool.tile([128, ITILE], f32, tag="recipB")
                    nc.vector.reciprocal_approx_fast(out=recipB[:], in_=dB[:])
                    for cc in range(NCC):
                        ot = opool.tile([128, ITILE], f32, tag="ot")
                        nc.vector.tensor_mul(ot[:], ops[cc][:], recipB[:])
                        nc.sync.dma_start(out_d[cc * 128:(cc + 1) * 128, isl], ot[:])

    nc.compile()
    return nc, run_bass_kernel_spmd


def kernel(x, Wq, bq, Wk, bk, Wv, bv, alpha, trace=False, trace_kwargs=None):
    if "nc" not in _cache:
        _cache["nc"] = _build()
    nc, run_spmd = _cache["nc"]

    x = np.ascontiguousarray(np.asarray(x, dtype=np.float32)).reshape(B, C, HW)
    a = float(np.asarray(alpha, np.float32).reshape(-1)[0])
    wqt = np.ascontiguousarray(np.asarray(Wq, np.float32).T)
    wkt = np.ascontiguousarray(np.asarray(Wk, np.float32).T)
    wvt = _round_f32r(np.ascontiguousarray(np.asarray(Wv, np.float32).T * a))
    bq = np.asarray(bq, np.float32).reshape(CQ, 1)
    bk = np.asarray(bk, np.float32).reshape(CQ, 1)
    bv = (np.asarray(bv, np.float32) * a).reshape(1, C)

    in_maps = []
    for core in range(NCORES):
        b, ih = core // 2, core % 2
        xb = x[b]
        if ih:
            xb = np.ascontiguousarray(np.concatenate([xb[:, IH:], xb[:, :IH]], axis=1))
        in_maps.append({"x": xb, "xr": _round_f32r(xb), "wqt": wqt, "wkt": wkt,
                        "wvt": wvt, "bq": bq, "bk": bk, "bv": bv})

    kwargs = {}
    if trace:
        kwargs["trace"] = True
        kwargs.update(trace_kwargs or {})
    res = run_spmd(nc, in_maps, list(range(NCORES)), **kwargs)

    out = np.empty((B, C, HW), dtype=np.float32)
    for core in range(NCORES):
        b, ih = core // 2, core % 2
        out[b][:, ih * IH:(ih + 1) * IH] = res.results[core]["out"]
    if trace:
        return out.reshape(B, C, 64, 64), res
    return out.reshape(B, C, 64, 64)

